# revision 4
# baseline (speedup 1.0000x reference)
"""LSTM encoder (B=64, S=512, E=H=1024) on 8 trn2 NeuronCores — v2.

Strategy (vs. the v1 baseline, which spent ~55us/step on 7 single-dest
remote_dma_broadcast preps):
  - Tensor-parallel over the 4H gate dim: core j owns hidden channels
    128j..128j+127 (4 gates x 128 = 512 gate rows), full batch, full seq.
  - Phase 1: embedding gather + gx = W_ih' X^T for all tokens -> DRAM (bf16),
    bias folded into the psum->stage copy.
  - Phase 2: 512 sequential steps. The h-exchange is ONE 8-destination
    remote_dma_broadcast per step (measured ~1.3us round trip on HW vs ~9us
    per single-dest broadcast): every core receives sender j's h-slice at
    slot j ("sender-indexed slots"), which is expressible in SPMD via a
    gp.Switch(partition_id) with 8 bodies that differ only in the out_ap
    slot. Self-delivery also goes through the fabric, so all engines
    outside the Switch are core-independent.
  - All-tanh gates: host pre-scales i/f/o rows by 0.5 so sigmoid(x) =
    0.5*(tanh(x/2)+1) becomes uniform tanh; state kept as c2=2c, h2=2h
    (W_hh pre-scaled by 0.5 to absorb h2) which makes the cell update
    3 fused scalar_tensor_tensor ops:
        A  = (tf + 1) * c2
        Bv = (ti + 1) * tg
        c2' = 0.5*A + Bv
        thc = tanh(0.5 * c2')        (ACT, free scale)
        h2  = (to + 1) * thc
    Host divides the final h2/c2 outputs by 2.

Self-contained: hardcodes all shapes; host-side prep is numpy only.
"""

import sys

sys.path.insert(0, "/opt/trn_rl_repo")

import numpy as np
import ml_dtypes

import concourse.bass as bass
import concourse.bacc as bacc
import concourse.mybir as mybir

BF16 = ml_dtypes.bfloat16
AF = mybir.ActivationFunctionType
ALU = mybir.AluOpType
dt = mybir.dt

VOCAB, EMB, HID = 32000, 1024, 1024
B = 64
S = 512
CORES = 8
KC = 8             # contraction chunks of 128
NCHUNK = 4         # gate chunks per core; order i, f, o, g
G = NCHUNK * 128   # 512 gate rows per core
NT = 512           # tokens per phase-1 tile
TPT = NT // B      # timesteps per phase-1 tile (8)
# chunk -> pytorch gate block (i=0, f=1, g=2, o=3); our order: g, i, f, o
CHUNK_TO_BLOCK = [2, 0, 1, 3]
# scale on W_ih/bias rows per chunk (sigmoid->tanh halving; g unscaled)
CHUNK_IH_SCALE = [1.0, 0.5, 0.5, 0.5]


def build(nc_steps=S, exchange=True, fp8=True, repeat=1):
    nsteps = nc_steps
    TT = B * nsteps // NT
    assert B * nsteps % NT == 0

    nc = bacc.Bacc(None, target_bir_lowering=False)

    # ---- kernel I/O (per core) ----
    emb_d = nc.declare_dram_parameter("emb16", [VOCAB, EMB], dt.bfloat16, isOutput=False)
    idx_d = nc.declare_dram_parameter("idx", [TT, 128, NT // 16], dt.int16, isOutput=False)
    wih_d = nc.declare_dram_parameter("w_ih", [128, KC * G], dt.bfloat16, isOutput=False)
    wdt = dt.float8e4 if fp8 else dt.bfloat16
    whh_d = nc.declare_dram_parameter("w_hh", [128, KC * G], wdt, isOutput=False)
    ident_d = nc.declare_dram_parameter("ident", [128, 128], dt.bfloat16, isOutput=False)
    gbias_d = nc.declare_dram_parameter("gbias", [128, NCHUNK], dt.float32, isOutput=False)
    out_d = nc.declare_dram_parameter("out", [2, 128, B], dt.float32, isOutput=True)

    # ---- DRAM scratch ----
    gx_d = nc.dram_tensor("gx", [128, nsteps, NCHUNK * B], dt.bfloat16)
    bar_in = nc.dram_tensor("bar_in", [128, 4], dt.float32)
    bar_out = nc.dram_tensor("bar_out", [128, 4], dt.float32, addr_space="Shared")

    # ---- semaphores ----
    cc_sem = nc.alloc_semaphore("cc_sem")
    bar_sem = nc.alloc_semaphore("bar_sem")
    bardma_sem = nc.alloc_semaphore("bardma_sem")
    wload = nc.alloc_semaphore("wload")
    g_sem = [nc.alloc_semaphore("g_sem0"), nc.alloc_semaphore("g_sem1")]
    mm1 = nc.alloc_semaphore("mm1")
    cp_sem = nc.alloc_semaphore("cp_sem")
    st_sem = [nc.alloc_semaphore("st_sem0"), nc.alloc_semaphore("st_sem1")]
    gxd = [nc.alloc_semaphore("gxd0"), nc.alloc_semaphore("gxd1")]
    idm = nc.alloc_semaphore("idm")
    mmr = nc.alloc_semaphore("mmr")
    act_s = nc.alloc_semaphore("act_s")
    dve_s = nc.alloc_semaphore("dve_s")
    prep_s = nc.alloc_semaphore("prep_s")
    rsem = [nc.alloc_semaphore("rsem0"), nc.alloc_semaphore("rsem1")]
    lsem = [nc.alloc_semaphore("lsem0"), nc.alloc_semaphore("lsem1")]
    fin = nc.alloc_semaphore("fin")

    NIDX = NT // 16

    from contextlib import ExitStack

    with ExitStack() as ctx:
        sb = lambda name, shape, d: ctx.enter_context(nc.sbuf_tensor(name, shape, d))
        idx_sb = sb("idx_sb", [128, TT * NIDX], dt.int16)
        wih_sb = sb("wih_sb", [128, KC * G], dt.bfloat16)
        whh_sb = sb("whh_sb", [128, KC * G], wdt)
        ident_sb = sb("ident_sb", [128, 128], dt.bfloat16)
        gbias_sb = sb("gbias_sb", [128, NCHUNK], dt.float32)
        xt = [sb(f"xt{i}", [128, KC, NT], dt.bfloat16) for i in range(2)]
        stage = [sb(f"stage{i}", [128, TPT * NCHUNK * B], dt.bfloat16) for i in range(2)]
        hg = [sb(f"hg{i}", [128, CORES * B], dt.bfloat16) for i in range(2)]
        hsrc = sb("hsrc", [128, B], dt.bfloat16)
        gxt = [sb(f"gxt{i}", [128, NCHUNK * B], dt.bfloat16) for i in range(2)]
        sg = sb("sg", [128, NCHUNK * B], dt.float32)
        a_sb = sb("a_sb", [128, B], dt.float32)
        b_sb = sb("b_sb", [128, B], dt.float32)
        thc_sb = sb("thc_sb", [128, B], dt.float32)
        c_sb = sb("c_sb", [128, B], dt.float32)
        hout_sb = sb("hout_sb", [128, B], dt.float32)
        bar_sb = sb("bar_sb", [128, 4], dt.float32)
        psum = [
            ctx.enter_context(nc.psum_tensor(f"ps{i}", [128, 512], dt.float32))
            for i in range(8)
        ]
        block = ctx.enter_context(nc.Block())

        sg_g = sg[:, 0 * B : 1 * B]
        sg_i = sg[:, 1 * B : 2 * B]
        sg_f = sg[:, 2 * B : 3 * B]
        sg_o = sg[:, 3 * B : 4 * B]

        # =========== SYNC engine ===========
        @block.sync
        def _(sy):
            sy.dma_start(
                out=idx_sb.ap().rearrange("p (t c) -> p t c", t=TT),
                in_=idx_d.ap().rearrange("t p c -> p t c"),
            ).then_inc(wload, 16)
            sy.dma_start(out=wih_sb[:, :], in_=wih_d[:, :]).then_inc(wload, 16)
            sy.dma_start(out=whh_sb[:, :], in_=whh_d[:, :]).then_inc(wload, 16)
            sy.dma_start(out=ident_sb[:, :], in_=ident_d[:, :]).then_inc(wload, 16)
            sy.dma_start(out=gbias_sb[:, :], in_=gbias_d[:, :]).then_inc(wload, 16)

            # warmup stores (tiles 0,1), then per-step prefetch + rolling stores
            def store_tile(tau):
                sy.wait_ge(cp_sem, 4 * tau + 4)
                sy.dma_start(
                    out=gx_d[:, TPT * tau : TPT * (tau + 1), :],
                    in_=stage[tau % 2].ap().rearrange("p (t e) -> p t e", t=TPT),
                ).then_inc(st_sem[tau % 2], 16)

            for tau in range(min(2, TT)):
                store_tile(tau)
            if not exchange:
                for tau in range(2, TT):
                    store_tile(tau)
            sy.dma_start(out=gxt[0][:, :], in_=gx_d[:, 0, :]).then_inc(gxd[0], 16)
            if nsteps > 1:
                sy.dma_start(out=gxt[1][:, :], in_=gx_d[:, 1, :]).then_inc(gxd[1], 16)
            for t in range(2, nsteps * repeat):
                sy.wait_ge(idm, t - 1)
                sy.dma_start(out=gxt[t % 2][:, :], in_=gx_d[:, t % nsteps, :]).then_inc(gxd[t % 2], 16)
                if exchange and t % 8 == 7 and t // 8 + 2 < TT:
                    store_tile(t // 8 + 2)

            # final outputs: h2 and c2 (host halves them)
            sy.wait_ge(dve_s, 1 + 4 * nsteps * repeat)
            sy.dma_start(out=out_d[0, :, :], in_=hout_sb[:, :]).then_inc(fin, 16)
            sy.dma_start(out=out_d[1, :, :], in_=c_sb[:, :]).then_inc(fin, 16)
            sy.wait_ge(fin, 32)

        # =========== GPSIMD: barrier, gathers, h broadcast ===========
        @block.gpsimd
        def _(gp):
            gp.memset(bar_sb[:, :], 0.0).then_inc(bar_sem, 1)
            gp.wait_ge(bar_sem, 1)
            gp.dma_start(out=bar_in[:, :], in_=bar_sb[:, :]).then_inc(bardma_sem, 16)
            gp.wait_ge(bardma_sem, 16)
            gp.collective_compute(
                "AllReduce",
                mybir.AluOpType.add,
                ins=[bar_in.ap().opt()],
                outs=[bar_out.ap().opt()],
                replica_groups=[list(range(CORES))],
            ).then_inc(cc_sem, 1)

            # embedding gathers: warmup tiles here; rest interleaved below
            gp.wait_ge(wload, 80)

            def gather_tile(tau):
                gp.dma_gather(
                    out_ap=xt[tau % 2][:, :, :],
                    in_ap=emb_d[:, :],
                    idxs_ap=idx_sb[:, NIDX * tau : NIDX * (tau + 1)],
                    num_idxs=NT,
                    num_idxs_reg=NT,
                    elem_size=EMB,
                    transpose=True,
                ).then_inc(g_sem[tau % 2], 16)

            for tau in range(min(2, TT)):
                gather_tile(tau)
            if exchange:
                if TT > 2:
                    gp.wait_ge(mm1, 4)
                    gather_tile(2)
            else:
                for tau in range(2, TT):
                    gp.wait_ge(mm1, 4 * (tau - 2) + 4)
                    gather_tile(tau)

            # phase-2 h broadcast: one 8-dest bcast per step, slot = my core id
            gp.wait_ge(cc_sem, 1)
            RD = [(0, d) for d in range(CORES)]
            pid_reg = gp.partition_id()
            for j in (gp.Switch(pid_reg, CORES) if exchange else []):
                for t in range(nsteps * repeat - 1):
                    po = (t + 1) % 2
                    gp.remote_dma_broadcast(
                        out_ap=hg[po][:, B * j : B * (j + 1)],
                        in_ap=hsrc[:, :],
                        remote_sem=rsem[po],
                        local_sem=lsem[po],
                        rdests=RD,
                    ).then_inc(prep_s, 1)
                    gp.wait_ge(prep_s, t + 1)
                    gp.wait_ge(dve_s, 1 + 4 * t + 4)  # h2(t) in hsrc
                    gp.trigger_dma(count=1)
                    if t % 8 == 0 and t // 8 + 3 < TT:
                        tau = t // 8 + 3
                        gp.wait_ge(mm1, 4 * (tau - 2) + 4)
                        gather_tile(tau)

        # =========== TENSOR engine ===========
        @block.tensor
        def _(te):
            te.wait_ge(wload, 80)

            def tile_mms(tau, cb, ks):
                for k in ks:
                    mm = te.matmul(
                        psum[4 + cb][:, :],
                        lhsT=wih_sb[:, G * k + 128 * cb : G * k + 128 * (cb + 1)],
                        rhs=xt[tau % 2][:, k, :],
                        start=(k == 0),
                        stop=(k == KC - 1),
                    )
                return mm

            # ---- warmup: tiles 0, 1 (sequential, single-buffered psum 4..7) ----
            for tau in range(min(2, TT)):
                te.wait_ge(g_sem[tau % 2], 16 * (tau // 2 + 1))
                for cb in range(NCHUNK):
                    if tau >= 1:
                        te.wait_ge(cp_sem, 4 * (tau - 1) + cb + 1)
                    tile_mms(tau, cb, range(KC)).then_inc(mm1, 1)
            if not exchange:
                for tau in range(2, TT):
                    te.wait_ge(g_sem[tau % 2], 16 * (tau // 2 + 1))
                    for cb in range(NCHUNK):
                        te.wait_ge(cp_sem, 4 * (tau - 1) + cb + 1)
                        tile_mms(tau, cb, range(KC)).then_inc(mm1, 1)

            # ---- phase 2, with tiles 2.. interleaved (quarter tile per step) ----
            for t in range(nsteps * repeat):
                P = t % 2
                te.wait_ge(gxd[P], 16 * (t // 2 + 1))
                if t >= 1:
                    te.wait_ge(act_s, 5 * (t - 1) + 4)  # gate ACTs of t-1 freed psum 0..3
                for cb in range(NCHUNK):
                    mm = te.matmul(
                        psum[cb][:, 0:B],
                        lhsT=ident_sb[:, :],
                        rhs=gxt[P][:, B * cb : B * (cb + 1)],
                        start=True,
                        stop=(t == 0),
                    )
                mm.then_inc(idm, 1)
                tau = t // 8 + 2
                if exchange and tau < TT:
                    m = t % 8
                    cb1, half = m // 2, m % 2
                    if m == 0:
                        te.wait_ge(g_sem[tau % 2], 16 * (tau // 2 + 1))
                    if half == 0:
                        te.wait_ge(cp_sem, 4 * (tau - 1) + cb1 + 1)
                        tile_mms(tau, cb1, range(4))
                    else:
                        tile_mms(tau, cb1, range(4, KC)).then_inc(mm1, 1)
                if t >= 1:
                    if exchange:
                        te.wait_ge(rsem[P], 16 * ((t + 1) // 2))
                    for cb in range(NCHUNK):
                        for d in range(CORES):
                            mm = te.matmul(
                                psum[cb][:, 0:B],
                                lhsT=whh_sb[:, G * d + 128 * cb : G * d + 128 * (cb + 1)],
                                rhs=hg[P][:, B * d : B * (d + 1)],
                                start=False,
                                stop=(d == CORES - 1),
                            )
                        mm.then_inc(mmr, 1)

        # =========== SCALAR engine (ACT) ===========
        @block.scalar
        def _(sc):
            sc.wait_ge(wload, 80)
            # ---- phase 2: per-chunk tanh (pipelines under the MM stream) ----
            for t in range(nsteps * repeat):
                P = t % 2
                for cb in range(NCHUNK):
                    if t == 0:
                        sc.wait_ge(idm, 1)
                    else:
                        sc.wait_ge(mmr, 4 * (t - 1) + cb + 1)
                    sc.activation(
                        sg[:, B * cb : B * (cb + 1)],
                        psum[cb][:, 0:B],
                        AF.Tanh,
                        scale=(1.0 / 32.0) if fp8 else 1.0,
                    ).then_inc(act_s, 1)
                sc.wait_ge(dve_s, 1 + 4 * t + 3)  # c2'(t) written
                sc.activation(thc_sb[:, :], c_sb[:, :], AF.Tanh, scale=0.5).then_inc(
                    act_s, 1
                )

        # =========== VECTOR engine (DVE) ===========
        @block.vector
        def _(ve):
            ve.wait_ge(wload, 80)

            def copy_tile_chunk(tau, cb):
                ve.wait_ge(mm1, 4 * tau + cb + 1)
                if tau >= 2:
                    ve.wait_ge(st_sem[tau % 2], 16 * (tau // 2))
                src = psum[4 + cb].ap().rearrange("p (t b) -> p t b", t=TPT)
                dst = stage[tau % 2].ap().rearrange(
                    "p (t e b) -> p t e b", t=TPT, e=NCHUNK
                )[:, :, cb, :]
                ve.tensor_scalar_add(dst, src, gbias_sb[:, cb : cb + 1]).then_inc(
                    cp_sem, 1
                )

            # warmup copies (tiles 0, 1)
            for tau in range(min(2, TT)):
                for cb in range(NCHUNK):
                    copy_tile_chunk(tau, cb)
            if not exchange:
                for tau in range(2, TT):
                    for cb in range(NCHUNK):
                        copy_tile_chunk(tau, cb)

            # ---- phase 2 ----
            ve.memset(c_sb[:, :], 0.0).then_inc(dve_s, 1)
            for t in range(nsteps * repeat):
                ve.wait_ge(act_s, 5 * t + 2)  # tanh g, i done
                ve.wait_ge(dve_s, max(1, 4 * t))  # b_sb WAR vs c2'(t-1)
                ve.scalar_tensor_tensor(
                    b_sb[:, :], sg_i, 1.0, sg_g, ALU.add, ALU.mult
                ).then_inc(dve_s, 1)
                ve.wait_ge(act_s, 5 * t + 3)  # tanh f done
                ve.scalar_tensor_tensor(
                    a_sb[:, :], sg_f, 1.0, c_sb[:, :], ALU.add, ALU.mult
                ).then_inc(dve_s, 1)
                ve.wait_ge(dve_s, 1 + 4 * t + 2)  # A, B written back
                ve.scalar_tensor_tensor(
                    c_sb[:, :], a_sb[:, :], 0.5, b_sb[:, :], ALU.mult, ALU.add
                ).then_inc(dve_s, 1)
                ve.wait_ge(act_s, 5 * t + 5)  # tanh o, thc done
                if t == nsteps * repeat - 1:
                    ve.scalar_tensor_tensor(
                        hout_sb[:, :], sg_o, 1.0, thc_sb[:, :], ALU.add, ALU.mult
                    ).then_inc(dve_s, 1)
                else:
                    if exchange and t >= 1:
                        ve.wait_ge(lsem[t % 2], 16 * ((t + 1) // 2))
                    ve.scalar_tensor_tensor(
                        hsrc[:, :], sg_o, 1.0, thc_sb[:, :], ALU.add, ALU.mult
                    ).then_inc(dve_s, 1)
                if exchange and t % 2 == 1 and t // 8 + 2 < TT:
                    copy_tile_chunk(t // 8 + 2, (t % 8) // 2)

    nc.compile()
    return nc


# ---------------------------------------------------------------------------
# host-side input prep
# ---------------------------------------------------------------------------

def prepare_in_maps(source, emb, W_ih, W_hh, b_ih, b_hh, nsteps=S, fp8=True):
    source = np.asarray(source)
    emb = np.asarray(emb, np.float32)
    W_ih = np.asarray(W_ih, np.float32)
    W_hh = np.asarray(W_hh, np.float32)
    b = np.asarray(b_ih, np.float32) + np.asarray(b_hh, np.float32)

    TT = B * nsteps // NT
    emb16 = emb.astype(BF16)
    ident = np.eye(128, dtype=BF16)

    idx = np.zeros([TT, 128, NT // 16], np.int16)
    j = np.arange(NT)
    tprime, bb = j // B, j % B
    for tau in range(TT):
        ids = source[bb, TPT * tau + tprime].astype(np.int16)
        wrapped = ids.reshape(NT // 16, 16).T
        idx[tau] = np.tile(wrapped, (8, 1))

    in_maps = []
    H = HID
    for jc in range(CORES):
        rows = np.concatenate(
            [
                np.arange(CHUNK_TO_BLOCK[cb] * H + 128 * jc,
                          CHUNK_TO_BLOCK[cb] * H + 128 * (jc + 1))
                for cb in range(NCHUNK)
            ]
        )
        scale_rows = np.repeat(np.array(CHUNK_IH_SCALE, np.float32), 128)[:, None]
        GS = 32.0 if fp8 else 1.0  # gate-domain upscale (fp8 subnormal dodge)
        Wi = W_ih[rows] * scale_rows * GS              # [512, 1024]
        Wh = W_hh[rows] * scale_rows * 0.5 * GS        # extra 0.5: h2 = 2h
        bi = b[rows] * scale_rows[:, 0] * GS

        wi4 = Wi.reshape(NCHUNK, 128, KC, 128)
        wih = np.transpose(wi4, (3, 2, 0, 1)).reshape(128, KC * G).astype(BF16)
        wh4 = Wh.reshape(NCHUNK, 128, KC, 128)
        WDT = ml_dtypes.float8_e4m3 if fp8 else BF16
        whh = np.transpose(wh4, (3, 2, 0, 1)).reshape(128, KC * G).astype(WDT)
        gbias = bi.reshape(NCHUNK, 128).T.copy().astype(np.float32)

        in_maps.append(
            {
                "emb16": emb16,
                "idx": idx,
                "w_ih": wih,
                "w_hh": whh,
                "ident": ident,
                "gbias": gbias,
            }
        )
    return in_maps


_BUILD_CACHE = {}


def _get_nc(nsteps=S, exchange=True, repeat=1):
    key = (nsteps, exchange, repeat)
    if key not in _BUILD_CACHE:
        _BUILD_CACHE[key] = build(nsteps, exchange, repeat=repeat)
    return _BUILD_CACHE[key]


def kernel(source, emb, W_ih, W_hh, b_ih, b_hh, _trace=False):
    from concourse.bass_utils import run_bass_kernel_spmd

    nc = _get_nc()
    in_maps = prepare_in_maps(source, emb, W_ih, W_hh, b_ih, b_hh)
    res = run_bass_kernel_spmd(nc, in_maps, core_ids=list(range(CORES)), trace=_trace)
    outs = [res.results[i]["out"] for i in range(CORES)]
    h = np.concatenate([o[0].T for o in outs], axis=1) * 0.5  # h2 -> h
    c = np.concatenate([o[1].T for o in outs], axis=1) * 0.5  # c2 -> c
    out = np.stack([h, c]).astype(np.float32)
    if _trace:
        return out, res
    return out


# ---------------------------------------------------------------------------
# dev: multi-core simulation on a reduced problem
# ---------------------------------------------------------------------------

_M = [0, 1, 2, 3, 6, 7, 4, 5]


def _fake_maps():
    from concourse import bass_interp, libnrt

    fake_map = {(d, i): _M[i] for d in range(16) for i in range(8)}
    libnrt.get_trn2_nc_mapping = lambda: fake_map
    libnrt.nc_to_real_nc = lambda dev, i: fake_map[(dev, i)]
    bass_interp.nc_to_real_nc = libnrt.nc_to_real_nc
    bass_interp.pnc_id_to_device_and_real_nc_index = (
        lambda core_id: (core_id // 8, fake_map[(core_id // 8, core_id % 8)])
    )
    fake_rid = {d: d for d in range(16)}
    libnrt.get_device_id_to_routing_id_mapping = lambda: fake_rid
    bass_interp.get_device_id_to_routing_id_mapping = lambda: fake_rid


def _simulate(nsteps=8):
    from concourse import bass_interp

    _fake_maps()
    rng = np.random.default_rng(0)
    source = rng.integers(0, VOCAB, (B, nsteps)).astype(np.int32)
    emb = rng.standard_normal((VOCAB, EMB), np.float32)
    W_ih = (rng.standard_normal((4 * HID, EMB), np.float32) / np.sqrt(EMB)).astype(np.float32)
    W_hh = (rng.standard_normal((4 * HID, HID), np.float32) / np.sqrt(HID)).astype(np.float32)
    b_ih = np.zeros(4 * HID, np.float32)
    b_hh = np.zeros(4 * HID, np.float32)

    nc = build(nsteps)
    in_maps = prepare_in_maps(source, emb, W_ih, W_hh, b_ih, b_hh, nsteps)

    sim = bass_interp.MultiCoreSim(nc, CORES)
    pid_name = nc.partition_id_tensor.name if nc.partition_id_tensor else None
    for i in range(CORES):
        for k, v in in_maps[i].items():
            sim.cores[i].tensor(k)[:] = v
        if pid_name:
            sim.cores[i].tensor(pid_name)[:] = np.array([[i]], np.uint32)
    sim.simulate()

    outs = [
        np.array(sim.cores[i].mem_tensor("out")).reshape(2, 128, B)
        for i in range(CORES)
    ]
    h = np.concatenate([o[0].T for o in outs], axis=1) * 0.5
    c = np.concatenate([o[1].T for o in outs], axis=1) * 0.5
    actual = np.stack([h, c])

    X = emb[source]
    hh = np.zeros((B, HID), np.float32)
    cc = np.zeros((B, HID), np.float32)
    for t in range(nsteps):
        gates = X[:, t, :] @ W_ih.T + hh @ W_hh.T + b_ih + b_hh
        i_, f_, g_, o_ = np.split(gates, 4, axis=-1)
        i_ = 1 / (1 + np.exp(-i_))
        f_ = 1 / (1 + np.exp(-f_))
        g_ = np.tanh(g_)
        o_ = 1 / (1 + np.exp(-o_))
        cc = f_ * cc + i_ * g_
        hh = o_ * np.tanh(cc)
    expected = np.stack([hh, cc])
    err = np.abs(actual - expected).max() / np.abs(expected).max()
    times = [sim.cores[i].time for i in range(CORES)]
    print(f"sim nsteps={nsteps} absmax_rel_err={err:.3e} sim_time_ns={max(times)}")
    return err


if __name__ == "__main__":
    ns = int(sys.argv[1]) if len(sys.argv) > 1 else 8
    _simulate(ns)


# revision 5
# speedup vs baseline: 1.3751x; 1.3751x over previous
"""LSTM encoder (B=64, S=512, E=H=1024) on 8 trn2 NeuronCores — v2.

Strategy (vs. the v1 baseline, which spent ~55us/step on 7 single-dest
remote_dma_broadcast preps):
  - Tensor-parallel over the 4H gate dim: core j owns hidden channels
    128j..128j+127 (4 gates x 128 = 512 gate rows), full batch, full seq.
  - Phase 1: embedding gather + gx = W_ih' X^T for all tokens -> DRAM (bf16),
    bias folded into the psum->stage copy.
  - Phase 2: 512 sequential steps. The h-exchange is ONE 8-destination
    remote_dma_broadcast per step (measured ~1.3us round trip on HW vs ~9us
    per single-dest broadcast): every core receives sender j's h-slice at
    slot j ("sender-indexed slots"), which is expressible in SPMD via a
    gp.Switch(partition_id) with 8 bodies that differ only in the out_ap
    slot. Self-delivery also goes through the fabric, so all engines
    outside the Switch are core-independent.
  - All-tanh gates: host pre-scales i/f/o rows by 0.5 so sigmoid(x) =
    0.5*(tanh(x/2)+1) becomes uniform tanh; state kept as c2=2c, h2=2h
    (W_hh pre-scaled by 0.5 to absorb h2) which makes the cell update
    3 fused scalar_tensor_tensor ops:
        A  = (tf + 1) * c2
        Bv = (ti + 1) * tg
        c2' = 0.5*A + Bv
        thc = tanh(0.5 * c2')        (ACT, free scale)
        h2  = (to + 1) * thc
    Host divides the final h2/c2 outputs by 2.

Self-contained: hardcodes all shapes; host-side prep is numpy only.
"""

import sys

sys.path.insert(0, "/opt/trn_rl_repo")

import numpy as np
import ml_dtypes

import concourse.bass as bass
import concourse.bacc as bacc
import concourse.mybir as mybir

BF16 = ml_dtypes.bfloat16
AF = mybir.ActivationFunctionType
ALU = mybir.AluOpType
dt = mybir.dt

VOCAB, EMB, HID = 32000, 1024, 1024
B = 64
S = 512
CORES = 8
KC = 8             # contraction chunks of 128
NCHUNK = 4         # gate chunks per core; order i, f, o, g
G = NCHUNK * 128   # 512 gate rows per core
NT = 512           # tokens per phase-1 tile
TPT = NT // B      # timesteps per phase-1 tile (8)
# chunk -> pytorch gate block (i=0, f=1, g=2, o=3); our order: g, i, f, o
CHUNK_TO_BLOCK = [2, 0, 1, 3]
# scale on W_ih/bias rows per chunk (sigmoid->tanh halving; g unscaled)
CHUNK_IH_SCALE = [1.0, 0.5, 0.5, 0.5]


def build(nc_steps=S, exchange=True, fp8=True, repeat=1):
    nsteps = nc_steps
    TT = B * nsteps // NT
    assert B * nsteps % NT == 0

    nc = bacc.Bacc(None, target_bir_lowering=False)

    # ---- kernel I/O (per core) ----
    emb_d = nc.declare_dram_parameter("emb16", [VOCAB, EMB], dt.bfloat16, isOutput=False)
    idx_d = nc.declare_dram_parameter("idx", [TT, 128, NT // 16], dt.int16, isOutput=False)
    wih_d = nc.declare_dram_parameter("w_ih", [128, KC * G], dt.bfloat16, isOutput=False)
    wdt = dt.float8e4 if fp8 else dt.bfloat16
    whh_d = nc.declare_dram_parameter("w_hh", [128, KC * G], wdt, isOutput=False)
    ident_d = nc.declare_dram_parameter("ident", [128, 128], dt.bfloat16, isOutput=False)
    gbias_d = nc.declare_dram_parameter("gbias", [128, NCHUNK], dt.float32, isOutput=False)
    out_d = nc.declare_dram_parameter("out", [2, 128, B], dt.float32, isOutput=True)

    # ---- DRAM scratch ----
    gx_d = nc.dram_tensor("gx", [128, nsteps, NCHUNK * B], dt.bfloat16)
    bar_in = nc.dram_tensor("bar_in", [128, 4], dt.float32)
    bar_out = nc.dram_tensor("bar_out", [128, 4], dt.float32, addr_space="Shared")

    # ---- semaphores ----
    cc_sem = nc.alloc_semaphore("cc_sem")
    bar_sem = nc.alloc_semaphore("bar_sem")
    bardma_sem = nc.alloc_semaphore("bardma_sem")
    wload = nc.alloc_semaphore("wload")
    g_sem = [nc.alloc_semaphore("g_sem0"), nc.alloc_semaphore("g_sem1")]
    mm1 = nc.alloc_semaphore("mm1")
    cp_sem = nc.alloc_semaphore("cp_sem")
    st_sem = [nc.alloc_semaphore("st_sem0"), nc.alloc_semaphore("st_sem1")]
    gxd = [nc.alloc_semaphore("gxd0"), nc.alloc_semaphore("gxd1")]
    idm = nc.alloc_semaphore("idm")
    mmr = nc.alloc_semaphore("mmr")
    act_s = nc.alloc_semaphore("act_s")
    dve_s = nc.alloc_semaphore("dve_s")
    prep_s = nc.alloc_semaphore("prep_s")
    rsem = [nc.alloc_semaphore("rsem0"), nc.alloc_semaphore("rsem1")]
    lsem = [nc.alloc_semaphore("lsem0"), nc.alloc_semaphore("lsem1")]
    fin = nc.alloc_semaphore("fin")

    NIDX = NT // 16

    from contextlib import ExitStack

    with ExitStack() as ctx:
        sb = lambda name, shape, d: ctx.enter_context(nc.sbuf_tensor(name, shape, d))
        idx_sb = sb("idx_sb", [128, TT * NIDX], dt.int16)
        wih_sb = sb("wih_sb", [128, KC * G], dt.bfloat16)
        whh_sb = sb("whh_sb", [128, KC * G], wdt)
        ident_sb = sb("ident_sb", [128, 128], dt.bfloat16)
        gbias_sb = sb("gbias_sb", [128, NCHUNK], dt.float32)
        xt = [sb(f"xt{i}", [128, KC, NT], dt.bfloat16) for i in range(2)]
        stage = [sb(f"stage{i}", [128, TPT * NCHUNK * B], dt.bfloat16) for i in range(2)]
        hg = [sb(f"hg{i}", [128, CORES * B], wdt) for i in range(2)]
        hsrc = sb("hsrc", [128, B], wdt)
        gxt = [sb(f"gxt{i}", [128, NCHUNK * B], dt.bfloat16) for i in range(2)]
        sg = sb("sg", [128, NCHUNK * B], dt.float32)
        a_sb = sb("a_sb", [128, B], dt.float32)
        b_sb = sb("b_sb", [128, B], dt.float32)
        thc_sb = sb("thc_sb", [128, B], dt.float32)
        c_sb = sb("c_sb", [128, B], dt.float32)
        hout_sb = sb("hout_sb", [128, B], dt.float32)
        bar_sb = sb("bar_sb", [128, 4], dt.float32)
        psum = [
            ctx.enter_context(nc.psum_tensor(f"ps{i}", [128, 512], dt.float32))
            for i in range(8)
        ]
        block = ctx.enter_context(nc.Block())

        sg_g = sg[:, 0 * B : 1 * B]
        sg_i = sg[:, 1 * B : 2 * B]
        sg_f = sg[:, 2 * B : 3 * B]
        sg_o = sg[:, 3 * B : 4 * B]

        # =========== SYNC engine ===========
        @block.sync
        def _(sy):
            sy.dma_start(
                out=idx_sb.ap().rearrange("p (t c) -> p t c", t=TT),
                in_=idx_d.ap().rearrange("t p c -> p t c"),
            ).then_inc(wload, 16)
            sy.dma_start(out=wih_sb[:, :], in_=wih_d[:, :]).then_inc(wload, 16)
            sy.dma_start(out=whh_sb[:, :], in_=whh_d[:, :]).then_inc(wload, 16)
            sy.dma_start(out=ident_sb[:, :], in_=ident_d[:, :]).then_inc(wload, 16)
            sy.dma_start(out=gbias_sb[:, :], in_=gbias_d[:, :]).then_inc(wload, 16)

            # warmup stores (tiles 0,1), then per-step prefetch + rolling stores
            def store_tile(tau):
                sy.wait_ge(cp_sem, 4 * tau + 4)
                sy.dma_start(
                    out=gx_d[:, TPT * tau : TPT * (tau + 1), :],
                    in_=stage[tau % 2].ap().rearrange("p (t e) -> p t e", t=TPT),
                ).then_inc(st_sem[tau % 2], 16)

            for tau in range(min(2, TT)):
                store_tile(tau)
            if not exchange:
                for tau in range(2, TT):
                    store_tile(tau)
            sy.dma_start(out=gxt[0][:, :], in_=gx_d[:, 0, :]).then_inc(gxd[0], 16)
            if nsteps > 1:
                sy.dma_start(out=gxt[1][:, :], in_=gx_d[:, 1, :]).then_inc(gxd[1], 16)
            for t in range(2, nsteps * repeat):
                sy.wait_ge(idm, t - 1)
                sy.dma_start(out=gxt[t % 2][:, :], in_=gx_d[:, t % nsteps, :]).then_inc(gxd[t % 2], 16)
                if exchange and t % 8 == 7 and t // 8 + 2 < TT:
                    store_tile(t // 8 + 2)

            # final outputs: h2 and c2 (host halves them)
            sy.wait_ge(dve_s, 1 + 4 * nsteps * repeat)
            sy.dma_start(out=out_d[0, :, :], in_=hout_sb[:, :]).then_inc(fin, 16)
            sy.dma_start(out=out_d[1, :, :], in_=c_sb[:, :]).then_inc(fin, 16)
            sy.wait_ge(fin, 32)

        # =========== GPSIMD: barrier, gathers, h broadcast ===========
        @block.gpsimd
        def _(gp):
            gp.memset(bar_sb[:, :], 0.0).then_inc(bar_sem, 1)
            gp.wait_ge(bar_sem, 1)
            gp.dma_start(out=bar_in[:, :], in_=bar_sb[:, :]).then_inc(bardma_sem, 16)
            gp.wait_ge(bardma_sem, 16)
            gp.collective_compute(
                "AllReduce",
                mybir.AluOpType.add,
                ins=[bar_in.ap().opt()],
                outs=[bar_out.ap().opt()],
                replica_groups=[list(range(CORES))],
            ).then_inc(cc_sem, 1)

            # embedding gathers: warmup tiles here; rest interleaved below
            gp.wait_ge(wload, 80)

            def gather_tile(tau):
                gp.dma_gather(
                    out_ap=xt[tau % 2][:, :, :],
                    in_ap=emb_d[:, :],
                    idxs_ap=idx_sb[:, NIDX * tau : NIDX * (tau + 1)],
                    num_idxs=NT,
                    num_idxs_reg=NT,
                    elem_size=EMB,
                    transpose=True,
                ).then_inc(g_sem[tau % 2], 16)

            for tau in range(min(2, TT)):
                gather_tile(tau)
            if exchange:
                if TT > 2:
                    gp.wait_ge(mm1, 4)
                    gather_tile(2)
            else:
                for tau in range(2, TT):
                    gp.wait_ge(mm1, 4 * (tau - 2) + 4)
                    gather_tile(tau)

            # phase-2 h broadcast: one 8-dest bcast per step, slot = my core id
            gp.wait_ge(cc_sem, 1)
            RD = [(0, d) for d in range(CORES)]
            pid_reg = gp.partition_id()
            for j in (gp.Switch(pid_reg, CORES) if exchange else []):
                for t in range(nsteps * repeat - 1):
                    po = (t + 1) % 2
                    gp.remote_dma_broadcast(
                        out_ap=hg[po][:, B * j : B * (j + 1)],
                        in_ap=hsrc[:, :],
                        remote_sem=rsem[po],
                        local_sem=lsem[po],
                        rdests=RD,
                    ).then_inc(prep_s, 1)
                    gp.wait_ge(prep_s, t + 1)
                    gp.wait_ge(dve_s, 1 + 4 * t + 4)  # h2(t) in hsrc
                    gp.trigger_dma(count=1)
                    if t % 8 == 0 and t // 8 + 3 < TT:
                        tau = t // 8 + 3
                        gp.wait_ge(mm1, 4 * (tau - 2) + 4)
                        gather_tile(tau)

        # =========== TENSOR engine ===========
        @block.tensor
        def _(te):
            te.wait_ge(wload, 80)

            def tile_mms(tau, cb, ks):
                for k in ks:
                    mm = te.matmul(
                        psum[4 + cb][:, :],
                        lhsT=wih_sb[:, G * k + 128 * cb : G * k + 128 * (cb + 1)],
                        rhs=xt[tau % 2][:, k, :],
                        start=(k == 0),
                        stop=(k == KC - 1),
                    )
                return mm

            # ---- warmup: tiles 0, 1 (sequential, single-buffered psum 4..7) ----
            for tau in range(min(2, TT)):
                te.wait_ge(g_sem[tau % 2], 16 * (tau // 2 + 1))
                for cb in range(NCHUNK):
                    if tau >= 1:
                        te.wait_ge(cp_sem, 4 * (tau - 1) + cb + 1)
                    tile_mms(tau, cb, range(KC)).then_inc(mm1, 1)
            if not exchange:
                for tau in range(2, TT):
                    te.wait_ge(g_sem[tau % 2], 16 * (tau // 2 + 1))
                    for cb in range(NCHUNK):
                        te.wait_ge(cp_sem, 4 * (tau - 1) + cb + 1)
                        tile_mms(tau, cb, range(KC)).then_inc(mm1, 1)

            # ---- phase 2, with tiles 2.. interleaved (quarter tile per step) ----
            for t in range(nsteps * repeat):
                P = t % 2
                te.wait_ge(gxd[P], 16 * (t // 2 + 1))
                if t >= 1:
                    te.wait_ge(act_s, 5 * (t - 1) + 4)  # gate ACTs of t-1 freed psum 0..3
                for cb in range(NCHUNK):
                    mm = te.matmul(
                        psum[cb][:, 0:B],
                        lhsT=ident_sb[:, :],
                        rhs=gxt[P][:, B * cb : B * (cb + 1)],
                        start=True,
                        stop=(t == 0),
                    )
                mm.then_inc(idm, 1)
                tau = t // 8 + 2
                if exchange and tau < TT:
                    m = t % 8
                    cb1, half = m // 2, m % 2
                    if m == 0:
                        te.wait_ge(g_sem[tau % 2], 16 * (tau // 2 + 1))
                    if half == 0:
                        te.wait_ge(cp_sem, 4 * (tau - 1) + cb1 + 1)
                        tile_mms(tau, cb1, range(4))
                    else:
                        tile_mms(tau, cb1, range(4, KC)).then_inc(mm1, 1)
                if t >= 1:
                    if exchange:
                        te.wait_ge(rsem[P], 16 * ((t + 1) // 2))
                    for cb in range(NCHUNK):
                        for d in range(CORES):
                            mm = te.matmul(
                                psum[cb][:, 0:B],
                                lhsT=whh_sb[:, G * d + 128 * cb : G * d + 128 * (cb + 1)],
                                rhs=hg[P][:, B * d : B * (d + 1)],
                                start=False,
                                stop=(d == CORES - 1),
                            )
                        mm.then_inc(mmr, 1)

        # =========== SCALAR engine (ACT) ===========
        @block.scalar
        def _(sc):
            sc.wait_ge(wload, 80)
            # ---- phase 2: per-chunk tanh (pipelines under the MM stream) ----
            for t in range(nsteps * repeat):
                P = t % 2
                for cb in range(NCHUNK):
                    if t == 0:
                        sc.wait_ge(idm, 1)
                    else:
                        sc.wait_ge(mmr, 4 * (t - 1) + cb + 1)
                    sc.activation(
                        sg[:, B * cb : B * (cb + 1)],
                        psum[cb][:, 0:B],
                        AF.Tanh,
                        scale=(1.0 / 32.0) if fp8 else 1.0,
                    ).then_inc(act_s, 1)
                sc.wait_ge(dve_s, 1 + 4 * t + 3)  # c2'(t) written
                sc.activation(thc_sb[:, :], c_sb[:, :], AF.Tanh, scale=0.5).then_inc(
                    act_s, 1
                )

        # =========== VECTOR engine (DVE) ===========
        @block.vector
        def _(ve):
            ve.wait_ge(wload, 80)

            def copy_tile_chunk(tau, cb):
                ve.wait_ge(mm1, 4 * tau + cb + 1)
                if tau >= 2:
                    ve.wait_ge(st_sem[tau % 2], 16 * (tau // 2))
                src = psum[4 + cb].ap().rearrange("p (t b) -> p t b", t=TPT)
                dst = stage[tau % 2].ap().rearrange(
                    "p (t e b) -> p t e b", t=TPT, e=NCHUNK
                )[:, :, cb, :]
                ve.tensor_scalar_add(dst, src, gbias_sb[:, cb : cb + 1]).then_inc(
                    cp_sem, 1
                )

            # warmup copies (tiles 0, 1)
            for tau in range(min(2, TT)):
                for cb in range(NCHUNK):
                    copy_tile_chunk(tau, cb)
            if not exchange:
                for tau in range(2, TT):
                    for cb in range(NCHUNK):
                        copy_tile_chunk(tau, cb)

            # ---- phase 2 ----
            ve.memset(c_sb[:, :], 0.0).then_inc(dve_s, 1)
            for t in range(nsteps * repeat):
                ve.wait_ge(act_s, 5 * t + 2)  # tanh g, i done
                ve.wait_ge(dve_s, max(1, 4 * t))  # b_sb WAR vs c2'(t-1)
                ve.scalar_tensor_tensor(
                    b_sb[:, :], sg_i, 1.0, sg_g, ALU.add, ALU.mult
                ).then_inc(dve_s, 1)
                ve.wait_ge(act_s, 5 * t + 3)  # tanh f done
                ve.scalar_tensor_tensor(
                    a_sb[:, :], sg_f, 1.0, c_sb[:, :], ALU.add, ALU.mult
                ).then_inc(dve_s, 1)
                ve.wait_ge(dve_s, 1 + 4 * t + 2)  # A, B written back
                ve.scalar_tensor_tensor(
                    c_sb[:, :], a_sb[:, :], 0.5, b_sb[:, :], ALU.mult, ALU.add
                ).then_inc(dve_s, 1)
                ve.wait_ge(act_s, 5 * t + 5)  # tanh o, thc done
                if t == nsteps * repeat - 1:
                    ve.scalar_tensor_tensor(
                        hout_sb[:, :], sg_o, 1.0, thc_sb[:, :], ALU.add, ALU.mult
                    ).then_inc(dve_s, 1)
                else:
                    if exchange and t >= 1:
                        ve.wait_ge(lsem[t % 2], 16 * ((t + 1) // 2))
                    ve.scalar_tensor_tensor(
                        hsrc[:, :], sg_o, 1.0, thc_sb[:, :], ALU.add, ALU.mult
                    ).then_inc(dve_s, 1)
                if exchange and t % 2 == 1 and t // 8 + 2 < TT:
                    copy_tile_chunk(t // 8 + 2, (t % 8) // 2)

    nc.compile()
    return nc


# ---------------------------------------------------------------------------
# host-side input prep
# ---------------------------------------------------------------------------

def prepare_in_maps(source, emb, W_ih, W_hh, b_ih, b_hh, nsteps=S, fp8=True):
    source = np.asarray(source)
    emb = np.asarray(emb, np.float32)
    W_ih = np.asarray(W_ih, np.float32)
    W_hh = np.asarray(W_hh, np.float32)
    b = np.asarray(b_ih, np.float32) + np.asarray(b_hh, np.float32)

    TT = B * nsteps // NT
    emb16 = emb.astype(BF16)
    ident = np.eye(128, dtype=BF16)

    idx = np.zeros([TT, 128, NT // 16], np.int16)
    j = np.arange(NT)
    tprime, bb = j // B, j % B
    for tau in range(TT):
        ids = source[bb, TPT * tau + tprime].astype(np.int16)
        wrapped = ids.reshape(NT // 16, 16).T
        idx[tau] = np.tile(wrapped, (8, 1))

    in_maps = []
    H = HID
    for jc in range(CORES):
        rows = np.concatenate(
            [
                np.arange(CHUNK_TO_BLOCK[cb] * H + 128 * jc,
                          CHUNK_TO_BLOCK[cb] * H + 128 * (jc + 1))
                for cb in range(NCHUNK)
            ]
        )
        scale_rows = np.repeat(np.array(CHUNK_IH_SCALE, np.float32), 128)[:, None]
        GS = 32.0 if fp8 else 1.0  # gate-domain upscale (fp8 subnormal dodge)
        Wi = W_ih[rows] * scale_rows * GS              # [512, 1024]
        Wh = W_hh[rows] * scale_rows * 0.5 * GS        # extra 0.5: h2 = 2h
        bi = b[rows] * scale_rows[:, 0] * GS

        wi4 = Wi.reshape(NCHUNK, 128, KC, 128)
        wih = np.transpose(wi4, (3, 2, 0, 1)).reshape(128, KC * G).astype(BF16)
        wh4 = Wh.reshape(NCHUNK, 128, KC, 128)
        WDT = ml_dtypes.float8_e4m3 if fp8 else BF16
        whh = np.transpose(wh4, (3, 2, 0, 1)).reshape(128, KC * G).astype(WDT)
        gbias = bi.reshape(NCHUNK, 128).T.copy().astype(np.float32)

        in_maps.append(
            {
                "emb16": emb16,
                "idx": idx,
                "w_ih": wih,
                "w_hh": whh,
                "ident": ident,
                "gbias": gbias,
            }
        )
    return in_maps


_BUILD_CACHE = {}


def _get_nc(nsteps=S, exchange=True, repeat=1):
    key = (nsteps, exchange, repeat)
    if key not in _BUILD_CACHE:
        _BUILD_CACHE[key] = build(nsteps, exchange, repeat=repeat)
    return _BUILD_CACHE[key]


def kernel(source, emb, W_ih, W_hh, b_ih, b_hh, _trace=False):
    from concourse.bass_utils import run_bass_kernel_spmd

    nc = _get_nc()
    in_maps = prepare_in_maps(source, emb, W_ih, W_hh, b_ih, b_hh)
    res = run_bass_kernel_spmd(nc, in_maps, core_ids=list(range(CORES)), trace=_trace)
    outs = [res.results[i]["out"] for i in range(CORES)]
    h = np.concatenate([o[0].T for o in outs], axis=1) * 0.5  # h2 -> h
    c = np.concatenate([o[1].T for o in outs], axis=1) * 0.5  # c2 -> c
    out = np.stack([h, c]).astype(np.float32)
    if _trace:
        return out, res
    return out


# ---------------------------------------------------------------------------
# dev: multi-core simulation on a reduced problem
# ---------------------------------------------------------------------------

_M = [0, 1, 2, 3, 6, 7, 4, 5]


def _fake_maps():
    from concourse import bass_interp, libnrt

    fake_map = {(d, i): _M[i] for d in range(16) for i in range(8)}
    libnrt.get_trn2_nc_mapping = lambda: fake_map
    libnrt.nc_to_real_nc = lambda dev, i: fake_map[(dev, i)]
    bass_interp.nc_to_real_nc = libnrt.nc_to_real_nc
    bass_interp.pnc_id_to_device_and_real_nc_index = (
        lambda core_id: (core_id // 8, fake_map[(core_id // 8, core_id % 8)])
    )
    fake_rid = {d: d for d in range(16)}
    libnrt.get_device_id_to_routing_id_mapping = lambda: fake_rid
    bass_interp.get_device_id_to_routing_id_mapping = lambda: fake_rid


def _simulate(nsteps=8):
    from concourse import bass_interp

    _fake_maps()
    rng = np.random.default_rng(0)
    source = rng.integers(0, VOCAB, (B, nsteps)).astype(np.int32)
    emb = rng.standard_normal((VOCAB, EMB), np.float32)
    W_ih = (rng.standard_normal((4 * HID, EMB), np.float32) / np.sqrt(EMB)).astype(np.float32)
    W_hh = (rng.standard_normal((4 * HID, HID), np.float32) / np.sqrt(HID)).astype(np.float32)
    b_ih = np.zeros(4 * HID, np.float32)
    b_hh = np.zeros(4 * HID, np.float32)

    nc = build(nsteps)
    in_maps = prepare_in_maps(source, emb, W_ih, W_hh, b_ih, b_hh, nsteps)

    sim = bass_interp.MultiCoreSim(nc, CORES)
    pid_name = nc.partition_id_tensor.name if nc.partition_id_tensor else None
    for i in range(CORES):
        for k, v in in_maps[i].items():
            sim.cores[i].tensor(k)[:] = v
        if pid_name:
            sim.cores[i].tensor(pid_name)[:] = np.array([[i]], np.uint32)
    sim.simulate()

    outs = [
        np.array(sim.cores[i].mem_tensor("out")).reshape(2, 128, B)
        for i in range(CORES)
    ]
    h = np.concatenate([o[0].T for o in outs], axis=1) * 0.5
    c = np.concatenate([o[1].T for o in outs], axis=1) * 0.5
    actual = np.stack([h, c])

    X = emb[source]
    hh = np.zeros((B, HID), np.float32)
    cc = np.zeros((B, HID), np.float32)
    for t in range(nsteps):
        gates = X[:, t, :] @ W_ih.T + hh @ W_hh.T + b_ih + b_hh
        i_, f_, g_, o_ = np.split(gates, 4, axis=-1)
        i_ = 1 / (1 + np.exp(-i_))
        f_ = 1 / (1 + np.exp(-f_))
        g_ = np.tanh(g_)
        o_ = 1 / (1 + np.exp(-o_))
        cc = f_ * cc + i_ * g_
        hh = o_ * np.tanh(cc)
    expected = np.stack([hh, cc])
    err = np.abs(actual - expected).max() / np.abs(expected).max()
    times = [sim.cores[i].time for i in range(CORES)]
    print(f"sim nsteps={nsteps} absmax_rel_err={err:.3e} sim_time_ns={max(times)}")
    return err


if __name__ == "__main__":
    ns = int(sys.argv[1]) if len(sys.argv) > 1 else 8
    _simulate(ns)
